# revision 32
# baseline (speedup 1.0000x reference)
import os
import sys

import numpy as np

_CUT = int(os.environ.get("K_CUT", "99"))

sys.path.insert(0, "/opt/trn_rl_repo")

import ml_dtypes  # noqa: E402

import concourse.bacc as bacc  # noqa: E402
import concourse.bass as bass  # noqa: E402
import concourse.tile as tile  # noqa: E402
from concourse import mybir  # noqa: E402
from concourse.bass_utils import run_bass_kernel_spmd  # noqa: E402

C, H, W = 512, 64, 64
HW = H * W          # 4096
C8 = 64             # pos-att channels
NCORE = 8
IB = HW // NCORE    # 512 A-rows per core
CH = C // NCORE     # 64 channels per core
H2 = W2 = 32
PIX = H2 * W2       # 1024
SCALE = 32.0        # fp8 range scaling for the P matmul
F32 = mybir.dt.float32
BF16 = mybir.dt.bfloat16
FP8 = mybir.dt.float8e4
AX = mybir.AxisListType.X
OP = mybir.AluOpType
AF = mybir.ActivationFunctionType
PM = mybir.MatmulPerfMode

_BF = ml_dtypes.bfloat16


def _bcast(ap, pos, n):
    """Insert a stride-0 (broadcast) free dim of size n at free position pos."""
    a = [list(d) for d in ap.ap]
    a.insert(1 + pos, [0, n])
    return bass.AP(tensor=ap.tensor, offset=ap.offset, ap=a)


def _unit(ap):
    """Append a trailing unit free dim (for reduce outputs)."""
    a = [list(d) for d in ap.ap] + [[0, 1]]
    return bass.AP(tensor=ap.tensor, offset=ap.offset, ap=a)


def _fap(t, off, dims):
    """AP into tile t at free-offset off with free dims [[stride, n], ...]."""
    return bass.AP(tensor=t.tensor, offset=t.offset + off,
                   ap=[list(t.ap[0])] + [list(d) for d in dims])


def _build_real():
    nc = bacc.Bacc()

    xhw = nc.declare_dram_parameter("xhw", [C, HW], BF16, isOutput=False)
    xP = nc.declare_dram_parameter("xP", [C, IB], BF16, isOutput=False)
    x5T2 = nc.declare_dram_parameter("x5T2", [2, 128, 2, C], BF16,
                                     isOutput=False)
    xpack = nc.declare_dram_parameter("xpack", [128, 2048], BF16,
                                      isOutput=False)
    wposT = nc.declare_dram_parameter("wposT", [C, C8], BF16, isOutput=False)
    bpos = nc.declare_dram_parameter("bpos", [C8, 1], F32, isOutput=False)
    w3a = nc.declare_dram_parameter("w3a", [4, 128, 9, CH], BF16,
                                    isOutput=False)
    w3b2 = nc.declare_dram_parameter("w3b2", [128, 9, C], BF16,
                                     isOutput=False)
    b3 = nc.declare_dram_parameter("b3", [CH, 1], F32, isOutput=False)
    mpair = nc.declare_dram_parameter("mpair", [128, 128], BF16,
                                      isOutput=False)
    out_ext = nc.declare_dram_parameter("out", [CH, HW], BF16, isOutput=True)

    warm_in = nc.dram_tensor("warm_in", [NCORE, 64], BF16)
    warm_out = nc.dram_tensor("warm_out", [1, 64], BF16)
    p_dram = nc.dram_tensor("p_dram", [C, HW], BF16)
    p_rs = nc.dram_tensor("p_rs", [CH, HW], BF16)
    x12d = nc.dram_tensor("x12d", [C, PIX], BF16)
    x12rs = nc.dram_tensor("x12rs", [CH, PIX], BF16)
    bT_dram = nc.dram_tensor("bT_dram", [CH, PIX], BF16)

    groups = [list(range(NCORE))]
    taps = [(1, 1)] + [(kh, kw) for kh in range(3) for kw in range(3)
                       if (kh, kw) != (1, 1)]

    with tile.TileContext(nc) as tc, \
         tc.tile_pool(name="big", bufs=1) as big, \
         tc.tile_pool(name="sm", bufs=1) as sm, \
         tc.tile_pool(name="stg", bufs=4) as stg, \
         tc.tile_pool(name="stat", bufs=2) as stat:

        # Warm-up collective: absorbs comm-init barrier off the critical path.
        nc.gpsimd.collective_compute(
            "ReduceScatter", OP.add, replica_groups=groups,
            ins=[warm_in[:, :]], outs=[warm_out[:, :]])

        # ---------- loads (consumption order) ----------
        wp = []
        for k in range(4):
            t = sm.tile([128, C8], BF16, tag=f"wp{k}")
            nc.sync.dma_start(out=t[:, :], in_=wposT[k * 128:(k + 1) * 128, :])
            wp.append(t)
        bpos_sb = sm.tile([C8, 1], F32, tag="bpos")
        nc.sync.dma_start(out=bpos_sb[:, :], in_=bpos[:, :])
        xp = []
        for k in range(4):
            t = sm.tile([128, IB], BF16, tag=f"xp{k}")
            nc.sync.dma_start(out=t[:, :], in_=xP[k * 128:(k + 1) * 128, :])
            xp.append(t)
        xsb = [big.tile([128, HW], BF16, tag=f"xsb{k}", name=f"xsb{k}")
               for k in range(4)]
        for cc in range(4):
            for k in range(4):
                nc.sync.dma_start(
                    out=xsb[k][:, cc * 1024:(cc + 1) * 1024],
                    in_=xhw[k * 128:(k + 1) * 128, cc * 1024:(cc + 1) * 1024])
        x5t2 = []
        for p in range(2):
            t = sm.tile([128, 2, C], BF16, tag=f"x5t2_{p}")
            nc.sync.dma_start(out=t[:, :, :], in_=x5T2[p, :, :, :])
            x5t2.append(t)
        w3sb = []
        for k in range(4):
            t = sm.tile([128, 9, CH], BF16, tag=f"w3a{k}")
            nc.sync.dma_start(out=t[:, :, :], in_=w3a[k, :, :, :])
            w3sb.append(t)
        b3_sb = sm.tile([CH, 1], F32, tag="b3")
        nc.sync.dma_start(out=b3_sb[:, :], in_=b3[:, :])
        w3b_sb = sm.tile([128, 9, C], BF16, tag="w3b2")
        nc.sync.dma_start(out=w3b_sb[:, :, :], in_=w3b2[:, :, :])
        xpack_sb = sm.tile([128, 2048], BF16, tag="xpack")
        nc.sync.dma_start(out=xpack_sb[:, :], in_=xpack[:, :])
        mp_sb = sm.tile([128, 128], BF16, tag="mpair")
        nc.sync.dma_start(out=mp_sb[:, :], in_=mpair[:, :])

        A8 = [big.tile([128, 2, HW], FP8, tag=f"A8_{p}", name=f"A8_{p}")
              for p in range(2)]
        x5t8 = [sm.tile([128, 2, C], FP8, tag=f"x5t8_{p}", name=f"x5t8_{p}")
                for p in range(2)]

        # ---------- x3f = w_pos @ x_hw + b (C8, HW); x3b (C8, IB) ----------
        x3f = big.tile([C8, HW], BF16, tag="x3f")
        x3b = sm.tile([C8, IB], BF16, tag="x3b")
        with tc.tile_pool(name="psF", bufs=2, space="PSUM") as psF:
            pt = psF.tile([128, 512], F32, tag="psf")
            for k in range(4):
                nc.tensor.matmul(
                    pt[:C8, :], wp[k][:, :], xp[k][:, :],
                    start=(k == 0), stop=(k == 3))
            nc.vector.tensor_scalar_add(out=x3b[:, :], in0=pt[:C8, :],
                                        scalar1=bpos_sb[:, :])
            for njj in range(8):
                pt = psF.tile([128, 512], F32, tag="psf")
                for k in range(4):
                    nc.tensor.matmul(
                        pt[:C8, :], wp[k][:, :],
                        xsb[k][:, njj * 512:(njj + 1) * 512],
                        start=(k == 0), stop=(k == 3))
                nc.vector.tensor_scalar_add(
                    out=x3f[:, njj * 512:(njj + 1) * 512], in0=pt[:C8, :],
                    scalar1=bpos_sb[:, :])

        if _CUT < 2:
            nc.sync.dma_start(out=out_ext[:, :], in_=x3f[:, :])
            return nc

        # ---------- A rows + softmax -> A8 (fp8); x5 scale ----------
        with tc.tile_pool(name="psA", bufs=1, space="PSUM") as psA:
            for mi in range(4):
                pr, sub = divmod(mi, 2)
                ptH = [psA.tile([128, 2048], F32, tag="A0", name="ptA"),
                       psA.tile([128, 2048], F32, tag="A1", name="ptB")]
                mx8 = stat.tile([128, 8], F32, tag="mx8")
                for njj in range(8):
                    pv = ptH[njj // 4][:, (njj % 4) * 512:
                                       (njj % 4) * 512 + 512]
                    nc.tensor.matmul(
                        pv, x3b[:, mi * 128:(mi + 1) * 128],
                        x3f[:, njj * 512:(njj + 1) * 512],
                        start=True, stop=True)
                    nc.vector.reduce_max(
                        out=mx8[:, njj:njj + 1], in_=pv, axis=AX)
                mneg = stat.tile([128, 1], F32, tag="mneg")
                nc.vector.reduce_max(out=mneg[:, :], in_=mx8[:, :], axis=AX,
                                     negate=True)
                s8 = stat.tile([128, 8], F32, tag="s8")
                for njj in range(8):
                    nc.scalar.activation(
                        out=A8[pr][:, sub, njj * 512:(njj + 1) * 512],
                        in_=ptH[njj // 4][:, (njj % 4) * 512:
                                          (njj % 4) * 512 + 512],
                        func=AF.Exp, bias=mneg[:, :], scale=1.0,
                        accum_out=s8[:, njj:njj + 1])
                rsr = stat.tile([128, 1], F32, tag="rsr", bufs=4)
                nc.vector.reduce_sum(out=rsr[:, :], in_=s8[:, :], axis=AX)
                nc.vector.reciprocal(out=rsr[:, :], in_=rsr[:, :])
                nc.vector.tensor_scalar(
                    out=x5t8[pr][:, sub, :], in0=x5t2[pr][:, sub, :],
                    scalar1=rsr[:, :], scalar2=SCALE,
                    op0=OP.mult, op1=OP.mult)

        if _CUT < 3:
            nc.sync.dma_start(out=out_ext[:, :], in_=x3f[:, :])
            return nc

        with tc.tile_pool(name="psP", bufs=2, space="PSUM") as psP, \
             tc.tile_pool(name="psC", bufs=2, space="PSUM") as psC, \
             tc.tile_pool(name="psS", bufs=1, space="PSUM") as psS, \
             tc.tile_pool(name="psZ", bufs=1, space="PSUM") as psZ:

            # ---------- P partial (fp8 DoubleRow) -> staging -> one RS ------
            for njj in range(8):
                for mc in range(4):
                    pt = psP.tile([128, 512], F32, tag="P")
                    for p in range(2):
                        nc.tensor.matmul(
                            pt[:, :],
                            x5t8[p][:, :, mc * 128:(mc + 1) * 128],
                            A8[p][:, :, njj * 512:(njj + 1) * 512],
                            start=(p == 0), stop=(p == 1),
                            perf_mode=PM.DoubleRow)
                    st = stg.tile([128, 512], BF16, tag="pstg")
                    if (njj * 4 + mc) % 2 == 0:
                        nc.vector.tensor_copy(out=st[:, :], in_=pt[:, :])
                    else:
                        nc.scalar.activation(out=st[:, :], in_=pt[:, :],
                                             func=AF.Copy)
                    nc.sync.dma_start(
                        out=p_dram[mc * 128:(mc + 1) * 128,
                                   njj * 512:(njj + 1) * 512],
                        in_=st[:, :])
            nc.gpsimd.collective_compute(
                "ReduceScatter", OP.add, replica_groups=groups,
                ins=[p_dram[:, :]], outs=[p_rs[:, :]])

            if _CUT < 4:
                nc.sync.dma_start(out=out_ext[:, :], in_=x3f[:, :])
                return nc

            # ---------- c3 = conv3x3(x) stride2 -> (CH, 1024) ----------
            c3 = sm.tile([CH, PIX], BF16, tag="c3")
            for ohc in range(2):
                o0 = ohc * 16
                pt = psC.tile([128, 512], F32, tag="conv")
                first = True
                for ti, (kh, kw) in enumerate(taps):
                    oo0 = o0
                    ih0 = 2 * oo0 - 1 + kh
                    if ih0 < 0:
                        oo0 += 1
                        ih0 += 2
                    cnt_oh = (o0 + 16) - oo0
                    if kw < 1:
                        iw0, ow0, cnt_ow = 1, 1, 31
                    else:
                        iw0, ow0, cnt_ow = kw - 1, 0, 32
                    for k in range(4):
                        src = xsb[k]
                        rhs = bass.AP(
                            tensor=src.tensor,
                            offset=src.offset + ih0 * 64 + iw0,
                            ap=[list(src.ap[0]),
                                [128, cnt_oh], [2, cnt_ow]])
                        outv = pt[:CH, :].rearrange(
                            "p (a b) -> p a b", a=16)[
                            :, oo0 - o0:oo0 - o0 + cnt_oh,
                            ow0:ow0 + cnt_ow]
                        nc.tensor.matmul(
                            outv, w3sb[k][:, kh * 3 + kw, :], rhs,
                            start=first,
                            stop=(ti == len(taps) - 1 and k == 3))
                        first = False
                nc.vector.tensor_scalar_add(
                    out=c3[:, ohc * 512:(ohc + 1) * 512], in0=pt[:CH, :],
                    scalar1=b3_sb[:, :])

            # x4_3 = sigmoid(leaky_relu(c3)); transposes to tview layout
            x43 = sm.tile([CH, PIX], BF16, tag="x43")
            nc.scalar.activation(out=x43[:, :], in_=c3[:, :], func=AF.Lrelu,
                                 alpha=0.2)
            nc.scalar.activation(out=x43[:, :], in_=x43[:, :], func=AF.Sigmoid)
            tc3 = sm.tile([CH, PIX], BF16, tag="tc3")
            nc.vector.transpose(out=tc3[:, :], in_=c3[:, :])
            tx43 = sm.tile([CH, PIX], BF16, tag="tx43")
            nc.vector.transpose(out=tx43[:, :], in_=x43[:, :])

            def product8(ta, tb, dst_sm):
                """Per-channel (32x32) products, 8-way tile-packed.

                ta/tb in tview layout [32*chi + w, a*32 + clo].
                Output dst_sm (128, 512) bf16 softmaxed over the inner 32:
                dst_sm[32*j + a, (chi*8 + r)*32 + b] for c = chi*32 + r*4 + j.
                """
                # Row-tile pairs (chi) must land in different PSUM banks;
                # col tiles (jj) may share a bank at distinct partitions.
                pS = psS.tile([128, 1024], F32, tag="prod")
                for r in range(8):
                    for jj in range(4):
                        for chi in range(2):
                            clo = r * 4 + jj
                            lhsT = _fap(ta[chi * 32:chi * 32 + 32, :],
                                        clo, [[32, 32]])
                            rhs = _fap(tb[chi * 32:chi * 32 + 32, :],
                                       clo, [[32, 32]])
                            o = chi * 512 + r * 32
                            nc.tensor.matmul(
                                pS[jj * 32:jj * 32 + 32, o:o + 32],
                                lhsT, rhs, start=True, stop=True,
                                tile_position=(32 * chi, 32 * jj))
                # softmax over the inner 32 (range-safe, no max-sub)
                for chi in range(2):
                    nc.scalar.activation(
                        out=dst_sm[:, chi * 256:chi * 256 + 256],
                        in_=pS[:, chi * 512:chi * 512 + 256],
                        func=AF.Exp)
                dv = dst_sm.rearrange("p (c b) -> p c b", c=16)
                ssum = stat.tile([128, 16], F32, tag="ssum")
                nc.vector.reduce_sum(out=_unit(ssum[:, :]), in_=dv, axis=AX)
                nc.vector.reciprocal(out=ssum[:, :], in_=ssum[:, :])
                nc.vector.tensor_tensor(out=dv, in0=dv,
                                        in1=_bcast(ssum[:, :], 1, 32),
                                        op=OP.mult)

            if os.environ.get("K_SUB") == "a":
                nc.sync.dma_start(out=out_ext[:, :], in_=x3f[:, :])
                return nc

            # x3_2 = softmax(c3 @ x43^T) (overlaps the P-RS window)
            x32sb = sm.tile([128, 512], BF16, tag="x32sb")
            product8(tc3, tx43, x32sb)
            if os.environ.get("K_SUB") == "b":
                nc.sync.dma_start(out=out_ext[:, :], in_=x3f[:, :])
                return nc

            # tx32[32*chi + k, h3*32 + clo] = x32[c][h3, k]
            tx32 = sm.tile([CH, PIX], BF16, tag="tx32")
            for chi in range(2):
                for jj in range(4):
                    tin = _fap(x32sb[jj * 32:jj * 32 + 32, :],
                               chi * 256, [[32, 8], [1, 32]])
                    tout = _fap(tx32[chi * 32:chi * 32 + 32, :],
                                jj, [[4, 8], [32, 32]])
                    nc.vector.transpose(out=tout, in_=tin)

            if _CUT < 5:
                nc.sync.dma_start(out=out_ext[:, :], in_=x3f[:, :])
                return nc

            # ---------- x6 softmax + z + x11 (128-partition packed) --------
            p6 = big.tile([128, 2048], BF16, tag="p6")
            nc.sync.dma_start(out=p6[0:64, :], in_=p_rs[:, 0:2048])
            nc.sync.dma_start(out=p6[64:128, :], in_=p_rs[:, 2048:4096])
            s6 = stat.tile([128, 1], F32, tag="s6")
            nc.scalar.activation(out=p6[:, :], in_=p6[:, :], func=AF.Exp,
                                 scale=1.0 / SCALE, accum_out=s6[:, :])
            s6b = stat.tile([128, 1], BF16, tag="s6b")
            nc.vector.tensor_copy(out=s6b[:, :], in_=s6[:, :])
            ptZ = psZ.tile([128, 512], F32, tag="z")
            nc.tensor.matmul(ptZ[:, 0:1], mp_sb[:, :], s6b[:, :],
                             start=True, stop=True)
            r6 = stat.tile([128, 1], F32, tag="r6")
            nc.vector.reciprocal(out=r6[:, :], in_=ptZ[:, 0:1])
            z = big.tile([128, 2048], BF16, tag="z")
            nc.vector.scalar_tensor_tensor(
                out=z[:, :], in0=p6[:, :], scalar=r6[:, :],
                in1=xpack_sb[:, :], op0=OP.mult, op1=OP.add)
            nc.scalar.activation(out=z[:, :], in_=z[:, :], func=AF.Exp)
            zv = z.rearrange("p (h w) -> p h w", h=32)
            zs = stat.tile([128, 32], F32, tag="zs")
            nc.vector.reduce_sum(out=_unit(zs[:, :]), in_=zv, axis=AX)
            nc.vector.reciprocal(out=zs[:, :], in_=zs[:, :])
            x11 = big.tile([128, 2048], BF16, tag="x11")
            xv11 = x11.rearrange("p (h w) -> p h w", h=32)
            nc.vector.tensor_tensor(out=xv11, in0=zv,
                                    in1=_bcast(zs[:, :], 1, 64), op=OP.mult)

            if _CUT < 6:
                nc.sync.dma_start(out=out_ext[:, :], in_=x3f[:, :])
                return nc

            # ---------- x1_2 partial conv (row-packed) + RS ----------
            x11a = x11[0:64, :]
            x11b = x11[64:128, :]
            # halo: copy x11 row 31 to partitions 64..127 so the ohc1
            # stream reads only the upper partition half
            x11h = sm.tile([128, 64], BF16, tag="x11h")
            nc.sync.dma_start(out=x11h[64:128, :],
                              in_=x11a[:, 31 * 64:32 * 64])
            for mc in range(4):
                pts = [psC.tile([128, 512], F32, tag="conv", name="ptx0"),
                       psC.tile([128, 512], F32, tag="conv", name="ptx1")]
                for ohc in range(2):
                    first = True
                    base = 64 * ohc
                    for ti, (kh, kw) in enumerate(taps):
                        if kw < 1:
                            iw0, ow0, cnt_ow = 1, 1, 31
                        else:
                            iw0, ow0, cnt_ow = kw - 1, 0, 32
                        lhsT = w3b_sb[base:base + 64, kh * 3 + kw,
                                      mc * 128:(mc + 1) * 128]
                        ov = pts[ohc][:, :].rearrange(
                            "p (a b) -> p a b", a=16)
                        last = (ti == len(taps) - 1)
                        if ohc == 0:
                            oo0 = 0
                            ih0 = -1 + kh
                            if ih0 < 0:
                                oo0 = 1
                                ih0 += 2
                            cnt_oh = 16 - oo0
                            rhs = _fap(x11a, ih0 * 64 + iw0,
                                       [[128, cnt_oh], [2, cnt_ow]])
                            nc.tensor.matmul(
                                ov[:, oo0:oo0 + cnt_oh, ow0:ow0 + cnt_ow],
                                lhsT, rhs, start=first, stop=last,
                                tile_position=(0, 0))
                            first = False
                        elif kh == 0:
                            # oh=16 reads halo row 31; oh 17..31 read x11b
                            rhs = _fap(x11h[64:128, :], iw0, [[2, cnt_ow]])
                            nc.tensor.matmul(
                                ov[:, 0, ow0:ow0 + cnt_ow],
                                lhsT, rhs, start=first, stop=False,
                                tile_position=(64, 0))
                            first = False
                            rhs = _fap(x11b, 1 * 64 + iw0,
                                       [[128, 15], [2, cnt_ow]])
                            nc.tensor.matmul(
                                ov[:, 1:16, ow0:ow0 + cnt_ow],
                                lhsT, rhs, start=False, stop=last,
                                tile_position=(64, 0))
                        else:
                            ih0 = 31 + kh - 32
                            rhs = _fap(x11b, ih0 * 64 + iw0,
                                       [[128, 16], [2, cnt_ow]])
                            nc.tensor.matmul(
                                ov[:, 0:16, ow0:ow0 + cnt_ow],
                                lhsT, rhs, start=first, stop=last,
                                tile_position=(64, 0))
                            first = False
                for ohc in range(2):
                    st = stg.tile([128, 512], BF16, tag="x12stg", bufs=2)
                    if ohc == 0:
                        nc.vector.tensor_copy(out=st[:, :], in_=pts[ohc][:, :])
                    else:
                        nc.scalar.activation(out=st[:, :], in_=pts[ohc][:, :],
                                             func=AF.Copy)
                    nc.sync.dma_start(
                        out=x12d[mc * 128:(mc + 1) * 128,
                                 ohc * 512:(ohc + 1) * 512],
                        in_=st[:, :])
            nc.gpsimd.collective_compute(
                "ReduceScatter", OP.add, replica_groups=groups,
                ins=[x12d[:, :]], outs=[x12rs[:, :]])

            if os.environ.get("K_SUB6") == "a":
                nc.sync.dma_start(out=out_ext[:, :], in_=x3f[:, :])
                return nc

            # ---------- products on x1_2 ----------
            x12 = sm.tile([CH, PIX], BF16, tag="x12")
            nc.sync.dma_start(out=x12[:, :], in_=x12rs[:, :])
            nc.vector.tensor_scalar_add(out=x12[:, :], in0=x12[:, :],
                                        scalar1=b3_sb[:, :])
            tx12 = sm.tile([CH, PIX], BF16, tag="tx12")
            nc.vector.transpose(out=tx12[:, :], in_=x12[:, :])

            x22sb = sm.tile([128, 512], BF16, tag="x22sb")
            product8(tx12, tc3, x22sb)
            x33sb = sm.tile([128, 512], BF16, tag="x33sb")
            product8(tx12, tx32, x33sb)

            if _CUT < 7:
                nc.sync.dma_start(out=out_ext[:, :], in_=x3f[:, :])
                return nc

            # ---------- x_f = relu(x3_3 + x2_2 + c3), back to c-layout ------
            nc.vector.tensor_tensor(out=x22sb[:, :], in0=x22sb[:, :],
                                    in1=x33sb[:, :], op=OP.add)
            # bounce via DRAM to restore (c, h*32+w) layout
            for jj in range(4):
                src = _fap(x22sb[jj * 32:jj * 32 + 32, :], 0,
                           [[256, 2], [32, 8], [1, 32]])
                dst = bass.AP(
                    tensor=bT_dram, offset=jj * 1024,
                    ap=[[32, 32], [32 * 1024, 2], [4 * 1024, 8], [1, 32]])
                nc.sync.dma_start(out=dst, in_=src)
            xfT = sm.tile([CH, PIX], BF16, tag="xfT")
            nc.sync.dma_start(out=xfT[:, :], in_=bT_dram[:, :])
            xf = sm.tile([CH, PIX], BF16, tag="xf")
            nc.vector.tensor_tensor(out=xf[:, :], in0=xfT[:, :], in1=c3[:, :],
                                    op=OP.add)
            nc.vector.tensor_scalar_max(out=xf[:, :], in0=xf[:, :],
                                        scalar1=0.0)

            # ---------- bilinear 2x upsample (w-pass then h-pass) ----------
            xv = xf.rearrange("p (h w) -> p h w", h=H2)
            uw = big.tile([CH, H2 * W], BF16, tag="uw")     # (64, 32h, 64w)
            uwv = uw.rearrange("p (h w) -> p h w", h=H2)
            nc.vector.tensor_copy(out=uwv[:, :, 0], in_=xv[:, :, 0])
            nc.vector.tensor_copy(out=uwv[:, :, W - 1], in_=xv[:, :, W2 - 1])
            dw = sm.tile([CH, H2 * (W2 - 1)], BF16, tag="dw")
            dwv = dw.rearrange("p (h w) -> p h w", h=H2)
            nc.gpsimd.tensor_tensor(out=dwv, in0=xv[:, :, 0:W2 - 1],
                                    in1=xv[:, :, 1:W2], op=OP.subtract)
            ovw = _fap(uw, 1, [[64, H2], [2, W2 - 1]])
            nc.vector.scalar_tensor_tensor(
                out=ovw, in0=dwv, scalar=-0.25, in1=xv[:, :, 0:W2 - 1],
                op0=OP.mult, op1=OP.add)
            evw = _fap(uw, 2, [[64, H2], [2, W2 - 1]])
            nc.vector.scalar_tensor_tensor(
                out=evw, in0=dwv, scalar=0.25, in1=xv[:, :, 1:W2],
                op0=OP.mult, op1=OP.add)
            # h-pass
            outsb = big.tile([CH, HW], BF16, tag="outsb")
            ov2 = outsb.rearrange("p (h w) -> p h w", h=H)
            nc.vector.tensor_copy(out=ov2[:, 0, :], in_=uwv[:, 0, :])
            nc.vector.tensor_copy(out=ov2[:, H - 1, :], in_=uwv[:, H2 - 1, :])
            dh = sm.tile([CH, (H2 - 1) * W], BF16, tag="dh")
            dhv = dh.rearrange("p (h w) -> p h w", h=H2 - 1)
            nc.gpsimd.tensor_tensor(out=dhv, in0=uwv[:, 0:H2 - 1, :],
                                    in1=uwv[:, 1:H2, :], op=OP.subtract)
            ovh = _fap(outsb, W, [[2 * W, H2 - 1], [1, W]])
            nc.vector.scalar_tensor_tensor(
                out=ovh, in0=dhv, scalar=-0.25, in1=uwv[:, 0:H2 - 1, :],
                op0=OP.mult, op1=OP.add)
            evh = _fap(outsb, 2 * W, [[2 * W, H2 - 1], [1, W]])
            nc.vector.scalar_tensor_tensor(
                out=evh, in0=dhv, scalar=0.25, in1=uwv[:, 1:H2, :],
                op0=OP.mult, op1=OP.add)

            nc.sync.dma_start(out=out_ext[:, :], in_=outsb[:, :])

    return nc


_NC_CACHE = {}
_LAST_IN_MAPS = None


def kernel(x, w_pos, b_pos, w3, b3):
    x = np.asarray(x, np.float32)
    w_pos = np.asarray(w_pos, np.float32)
    b_pos = np.asarray(b_pos, np.float32)
    w3 = np.asarray(w3, np.float32)
    b3 = np.asarray(b3, np.float32)

    x_ = x[0]                                   # (C, H, W)
    xhw = x_.reshape(C, HW)                     # i = h*W + w
    xwh = x_.transpose(0, 2, 1).reshape(C, HW)  # i = w*H + h
    bf = lambda a: np.ascontiguousarray(a).astype(_BF)  # noqa: E731

    xhw_bf = bf(xhw)
    wposT = bf(w_pos.reshape(C8, C).T)
    bpos = np.ascontiguousarray(b_pos.reshape(C8, 1))
    w3b_all = bf(w3.transpose(1, 2, 3, 0).reshape(C, 9, C))  # (cin, tap, cout)
    mpair = np.zeros((128, 128), np.float32)
    for p in range(128):
        mpair[p, p % 64] = 1.0
        mpair[p, p % 64 + 64] = 1.0
    mpair = bf(mpair)

    in_maps = []
    for m in range(NCORE):
        w3s = w3[m * CH:(m + 1) * CH]           # (CH, C, 3, 3)
        w3t = w3s.transpose(1, 2, 3, 0).reshape(C, 9, CH).reshape(
            4, 128, 9, CH)
        x5T = xhw[:, m * IB:(m + 1) * IB].T     # (IB, C)
        x5T2 = x5T.reshape(2, 2, 128, C).transpose(0, 2, 1, 3)
        # xpack[cc + 64*hh, hl*64 + w] = x[m*CH+cc, hh*32+hl, w]
        xpk = x_[m * CH:(m + 1) * CH].reshape(CH, 2, 32 * 64)
        xpk = xpk.transpose(1, 0, 2).reshape(128, 2048)
        # w3b2[q, tap, cout] = w3[cout, m*CH + q%64, kh, kw]
        w3b_m = w3b_all[m * CH:(m + 1) * CH]    # (CH, 9, C)
        w3b2 = np.concatenate([w3b_m, w3b_m], axis=0)
        in_maps.append({
            "xhw": xhw_bf,
            "xP": bf(xwh[:, m * IB:(m + 1) * IB]),
            "x5T2": bf(x5T2),
            "xpack": bf(xpk),
            "wposT": wposT,
            "bpos": bpos,
            "w3a": bf(w3t),
            "w3b2": np.ascontiguousarray(w3b2),
            "b3": np.ascontiguousarray(b3[m * CH:(m + 1) * CH].reshape(CH, 1)),
            "mpair": mpair,
        })

    global _LAST_IN_MAPS
    _LAST_IN_MAPS = in_maps
    if "nc" not in _NC_CACHE:
        nc_ = _build_real()
        nc_.finalize()
        _NC_CACHE["nc"] = nc_
    nc = _NC_CACHE["nc"]

    res = run_bass_kernel_spmd(nc, in_maps, core_ids=list(range(NCORE)))
    outs = [np.asarray(res.results[m]["out"], np.float32)
            for m in range(NCORE)]
    full = np.concatenate(outs, axis=0).reshape(1, C, H, W)
    return full
